# revision 1
# baseline (speedup 1.0000x reference)
"""ClusterMergeNet Trainium2 kernel.

Math: sim[b,i,j] (i<j) = sigmoid( sum_h W2c[h]*relu(A[b,i,h] + C[b,j,h] + b1c[h]) + b2c )
  with A = X @ W1c[:D], C = X @ W1c[D:]   (pair-MLP first layer decomposed).
Diagonal: sigmoid( sum_h W2s[h]*relu((X@W1s)[b,i,h] + b1s[h]) + b2s ).

Trick: W2c[h]*relu(z) = sign(W2c[h]) * relu(|W2c[h]|*z); |W2c| is folded into the
first-layer weights/bias on the host, so the device reduction over h is a plain
signed sum -> one-hot-column sign-vector matmuls accumulating on the PE.

Sharding (8 cores): batch b -> cores (2b, 2b+1). Role 0 computes row-tiles
{0,3}, role 1 row-tiles {1,2} of the 4x4 block triangle; uniform SPMD spans:
segment A = full j span [0,512), segment B = j span [256,512).
Host mirrors the strict upper triangle and fills the diagonal.

The walrus build on this image supports only ~1 semaphore wait per hardware
instruction, so the kernel is structured so Tile emits at most one wait each:
all inputs arrive via two packed DMAs, every consuming engine gets a fan-in
NOP per input DMA, DVE-gen and ACT-gen use separate tile pools (so slot reuse
never joins two engines), and output DMAs / the exit drain are preceded by
single-dependency NOPs.
"""

import threading

import numpy as np
import ml_dtypes

B, N, D, H = 4, 512, 128, 128
NCORES = 8
SEGS = ((0, 512), (256, 256))  # (j_offset, j_span) for segment A, B

# fp32 pack layout (columns)
C_XT = 0          # [0,512)   X_b.T
C_XIT = 512       # [512,768) own i-rows transposed
C_W1A = 768       # [768,896) W1c[:D]*|W2c|
C_W1B = 896       # [896,1024) W1c[D:]*|W2c|
C_W1S = 1024      # [1024,1152) W1s*|W2s|
C_BEFFC = 1152    # b1c*|W2c|
C_B1SE = 1153     # b1s*|W2s|
C_B2C = 1154      # b2c broadcast
C_B2S = 1155      # b2s broadcast
PK32_W = 1156
# bf16 pack: [0,64) = zz (col 32 = sign(W2c)), col 64 = sign(W2s)
PK16_W = 65

_lock = threading.Lock()
_cache = {}


def _build_nc():
    import concourse.bass as bass
    import concourse.mybir as mybir
    import concourse.tile as tile
    from concourse.tile import add_dep_helper

    fp32 = mybir.dt.float32
    bf16 = mybir.dt.bfloat16

    nc = bass.Bass("TRN2")
    pk32 = nc.dram_tensor("pk32", [128, PK32_W], fp32, kind="ExternalInput")
    pk16 = nc.dram_tensor("pk16", [128, PK16_W], bf16, kind="ExternalInput")
    va = nc.dram_tensor("va", [128, 512], fp32, kind="ExternalOutput")
    vb = nc.dram_tensor("vb", [128, 256], fp32, kind="ExternalOutput")
    dd = nc.dram_tensor("dd", [1, 256], fp32, kind="ExternalOutput")

    exit_prods = []

    def spnop(prod):
        n = nc.sync.nop(nofuse=True)
        add_dep_helper(n.ins, prod.ins, sync=True, reason="sp fanin")
        return n

    with tile.TileContext(nc) as tc:
        with (
            tc.tile_pool(name="singles", bufs=1) as singles,
            tc.tile_pool(name="mdve", bufs=6) as mdve,
            tc.tile_pool(name="mact", bufs=4) as mact,
            tc.tile_pool(name="vout", bufs=1) as vout,
            tc.tile_pool(name="pre_ps", bufs=1, space="PSUM") as pre_ps,
            tc.tile_pool(name="acc_ps", bufs=1, space="PSUM") as acc_ps,
        ):
            p32 = singles.tile([128, PK32_W], fp32, tag="p32")
            dma32 = nc.sync.dma_start(out=p32, in_=pk32[:, :])
            p16 = singles.tile([128, PK16_W], bf16, tag="p16")
            dma16 = nc.sync.dma_start(out=p16, in_=pk16[:, :])
            exit_prods += [dma32, dma16]

            # per-engine fan-in so no compute instruction joins two DMA queues
            for eng in (nc.tensor, nc.vector, nc.scalar):
                for dma in (dma32, dma16):
                    n = eng.nop(nofuse=True)
                    add_dep_helper(n.ins, dma.ins, sync=True, reason="in fanin")

            xt = p32[:, C_XT:C_XT + 512]
            xit = p32[:, C_XIT:C_XIT + 256]
            w1a = p32[:, C_W1A:C_W1A + 128]
            w1b = p32[:, C_W1B:C_W1B + 128]
            w1s = p32[:, C_W1S:C_W1S + 128]
            beffc = p32[:, C_BEFFC:C_BEFFC + 1]
            b1se = p32[:, C_B1SE:C_B1SE + 1]
            b2c = p32[:, C_B2C:C_B2C + 1]
            b2s = p32[:, C_B2S:C_B2S + 1]
            zz = p16[:, 0:64]
            zs = p16[:, 64:65]

            # --- precompute: abias = (w1a.T @ xit) + beffc   [H, 256] f32 ---
            apsum = pre_ps.tile([H, 256], fp32, tag="apsum")
            nc.tensor.matmul(apsum, w1a, xit, start=True, stop=True)
            abias = singles.tile([H, 256], fp32, tag="abias")
            nc.vector.tensor_scalar(out=abias, in0=apsum, scalar1=beffc,
                                    scalar2=None, op0=mybir.AluOpType.add)

            # --- c2 = (w1b.T @ xt)  [H, N] bf16 (bias lives in abias) ---
            cpsum = pre_ps.tile([H, N], fp32, tag="cpsum")
            nc.tensor.matmul(cpsum, w1b, xt, start=True, stop=True)
            c2 = singles.tile([H, N], bf16, tag="c2")
            nc.vector.tensor_copy(out=c2, in_=cpsum)

            # --- self-sim diagonal for own 256 i-rows ---
            spsum = pre_ps.tile([H, 256], fp32, tag="spsum")
            nc.tensor.matmul(spsum, w1s, xit, start=True, stop=True)
            ms = singles.tile([H, 256], bf16, tag="ms")
            nc.vector.tensor_scalar(out=ms, in0=spsum, scalar1=b1se,
                                    scalar2=0.0, op0=mybir.AluOpType.add,
                                    op1=mybir.AluOpType.max)
            dpsum = pre_ps.tile([1, 256], fp32, tag="dpsum")
            nc.tensor.matmul(dpsum, zs, ms, start=True, stop=True)
            dsig = singles.tile([1, 256], fp32, tag="dsig")
            act_d = nc.scalar.activation(out=dsig, in_=dpsum,
                                         func=mybir.ActivationFunctionType.Sigmoid,
                                         bias=b2s[0:1, :], scale=1.0)
            spnop(act_d)
            exit_prods.append(nc.sync.dma_start(out=dd[:, :], in_=dsig))

            # --- main: per-i fused add+relu then one-hot sign matvec on PE ---
            last_gen = last_mm = last_act = None
            for seg, (joff, sp) in enumerate(SEGS):
                ps = acc_ps.tile([128, sp], fp32, tag=f"ps{seg}")
                for i in range(128):
                    ig = seg * 128 + i
                    if i % 10 < 7:
                        m = mdve.tile([H, sp], bf16, tag="mdve")
                        last_gen = nc.vector.tensor_scalar(
                            out=m, in0=c2[:, joff:joff + sp],
                            scalar1=abias[:, ig:ig + 1], scalar2=0.0,
                            op0=mybir.AluOpType.add, op1=mybir.AluOpType.max)
                    else:
                        m = mact.tile([H, sp], bf16, tag="mact")
                        last_gen = nc.scalar.activation(
                            out=m, in_=c2[:, joff:joff + sp],
                            func=mybir.ActivationFunctionType.Relu,
                            bias=abias[:, ig:ig + 1], scale=1.0)
                    c, mc = i // 32, i % 32
                    last_mm = nc.tensor.matmul(
                        ps[32 * c:32 * c + 32, :],
                        zz[:, 32 - mc:64 - mc], m,
                        start=(mc == 0), stop=(mc == 31),
                        tile_position=(0, 32 * c))
                v = vout.tile([128, sp], fp32, tag=f"v{seg}")
                last_act = nc.scalar.activation(
                    out=v, in_=ps, func=mybir.ActivationFunctionType.Sigmoid,
                    bias=b2c, scale=1.0)
                spnop(last_act)
                out_ap = va if seg == 0 else vb
                exit_prods.append(nc.sync.dma_start(out=out_ap[:, :], in_=v))

            exit_prods += [last_gen, last_mm, last_act]

            # exit fan-in: give the tail drain an SP-observed clock
            for p in exit_prods:
                if p is not None:
                    spnop(p)
    return nc


def _split_waits(bir_bytes):
    """Post-pass: walrus on this image accepts ~1 sem wait per instruction.
    Hoist all-but-one wait of any multi-wait instruction onto same-engine
    NoOps inserted immediately before it (engine stalls on each in order --
    semantically identical, within the 1-wait limit)."""
    import json
    bir = json.loads(bir_bytes)
    counter = [0]

    def mknop(engine, wait, debug):
        counter[0] += 1
        return {
            "debug": debug,
            "engine": engine,
            "ins": [],
            "name": f"WSN-{counter[0]}",
            "opcode": "NoOp",
            "outs": [],
            "sync_info": {"on_update": [], "on_wait": [wait]},
        }

    def process(blocks):
        for blk in blocks:
            insts = blk.get("instructions")
            if not insts:
                continue
            out = []
            for ins in insts:
                si = ins.get("sync_info")
                ow = (si or {}).get("on_wait") or []
                if len(ow) > 1:
                    for w in ow[:-1]:
                        out.append(mknop(ins["engine"], w, ins.get("debug", 0)))
                    si["on_wait"] = [ow[-1]]
                out.append(ins)
            blk["instructions"] = out

    for func in bir.get("functions", []):
        process(func.get("blocks", []))
    return json.dumps(bir).encode()


def _get_nc():
    with _lock:
        if "nc" not in _cache:
            nc = _build_nc()
            orig = nc.to_json_bytes
            nc.to_json_bytes = lambda: _split_waits(orig())
            _cache["nc"] = nc
        return _cache["nc"]


def make_core_inputs(X, W1c, b1c, W2c, b2c, W1s, b1s, W2s, b2s):
    """Build the 8 per-core input maps (host-side weight folding)."""
    X = np.ascontiguousarray(np.asarray(X, np.float32))
    w2c = np.asarray(W2c, np.float32).reshape(-1)
    w2s = np.asarray(W2s, np.float32).reshape(-1)
    aw, sg = np.abs(w2c), np.sign(w2c).astype(np.float32)
    aws, sgs = np.abs(w2s), np.sign(w2s).astype(np.float32)
    W1c = np.asarray(W1c, np.float32)

    base = np.zeros((128, PK32_W), np.float32)
    base[:, C_W1A:C_W1A + 128] = W1c[:D] * aw[None, :]
    base[:, C_W1B:C_W1B + 128] = W1c[D:] * aw[None, :]
    base[:, C_W1S:C_W1S + 128] = np.asarray(W1s, np.float32) * aws[None, :]
    base[:, C_BEFFC] = np.asarray(b1c, np.float32) * aw
    base[:, C_B1SE] = np.asarray(b1s, np.float32) * aws
    base[:, C_B2C] = float(np.asarray(b2c).reshape(-1)[0])
    base[:, C_B2S] = float(np.asarray(b2s).reshape(-1)[0])

    p16 = np.zeros((128, PK16_W), ml_dtypes.bfloat16)
    p16[:, 32] = sg.astype(ml_dtypes.bfloat16)
    p16[:, 64] = sgs.astype(ml_dtypes.bfloat16)

    in_maps = []
    for c in range(NCORES):
        b, role = c // 2, c % 2
        ta, tb = (0, 3) if role == 0 else (1, 2)
        xi = np.concatenate([X[b, 128 * ta:128 * (ta + 1)],
                             X[b, 128 * tb:128 * (tb + 1)]], axis=0)  # [256, D]
        p32 = base.copy()
        p32[:, C_XT:C_XT + 512] = X[b].T
        p32[:, C_XIT:C_XIT + 256] = xi.T
        in_maps.append({"pk32": p32, "pk16": p16})
    return in_maps


def assemble(results, dtype=np.float32):
    """results: list of 8 dicts with va [128,512], vb [128,256], dd [1,256]."""
    sim = np.zeros((B, N, N), np.float32)
    for b in range(B):
        U = np.zeros((N, N), np.float32)
        diag = np.zeros(N, np.float32)
        for role in range(2):
            r = results[2 * b + role]
            ta, tb = (0, 3) if role == 0 else (1, 2)
            U[128 * ta:128 * (ta + 1), :] = r["va"]
            U[128 * tb:128 * (tb + 1), 256:] = r["vb"]
            d = np.asarray(r["dd"]).reshape(256)
            diag[128 * ta:128 * (ta + 1)] = d[:128]
            diag[128 * tb:128 * (tb + 1)] = d[128:]
        Ut = np.triu(U, 1)
        out = Ut + Ut.T
        np.fill_diagonal(out, diag)
        sim[b] = out
    return sim.astype(dtype)


def kernel(X, W1c, b1c, W2c, b2c, W1s, b1s, W2s, b2s):
    from concourse.bass_utils import run_bass_kernel_spmd

    nc = _get_nc()
    in_maps = make_core_inputs(X, W1c, b1c, W2c, b2c, W1s, b1s, W2s, b2s)
    res = run_bass_kernel_spmd(nc, in_maps, core_ids=list(range(NCORES)))
    return assemble(res.results, dtype=np.asarray(X).dtype)



# revision 22
# speedup vs baseline: 1.1076x; 1.1076x over previous
"""ClusterMergeNet Trainium2 kernel (v2: triangle-trim + fp8 DoubleRow).

Math: sim[b,i,j] (i<j) = sigmoid( sum_h W2c[h]*relu(A[b,i,h] + C[b,j,h] + b1c[h]) + b2c )
  with A = X @ W1c[:D], C = X @ W1c[D:]   (pair-MLP first layer decomposed).
Diagonal: sigmoid( sum_h W2s[h]*relu((X@W1s)[b,i,h] + b1s[h]) + b2s ).

Trick: W2c[h]*relu(z) = sign(W2c[h]) * relu(|W2c[h]|*S*z) / S; |W2c|*S is folded
into the first-layer weights/bias on the host (S=64 keeps fp8e4 values normal),
so the device reduction over h is a plain signed sum. The final sigmoid applies
scale 1/S.

Triangle trim: only j>i is needed. Rows are processed in 32-row blocks with
j-window [j0, j0+W), W = 512-64q for block-class q (8 classes per core). The two
cores of a batch split each 64-row stripe: half A takes rows [64q,64q+32) with
j0=64q, half B takes rows [64q+32,64q+64) with j0=64q+32 (the window then
reaches col 544; c2 cols 512..543 are exact zeros, host discards the junk).
Both halves execute the identical instruction stream (uniform SPMD).

Per-row reduction over h runs on the PE two ways:
 - bf16 singles: one-hot sign column (sliding window zz) matmul, 1 cyc/col.
 - fp8e4 DoubleRow pairs: two rows per matmul at 0.5 cyc/col via a [128,2,64]
   sign table t8 (t8[:,0,32]=sgn, t8[:,1,33]=sgn); lhsT = t8[:, :, 32-mc0 :
   64-mc0] places rows (mc0, mc0+1).
Generation (relu(c2[:,win] + abias[:,i])) is split across DVE (bf16 out, 4x
mode), ACT and Pool (fp8 out) per a build-time load-balance config.

The walrus build on this image supports only ~1 semaphore wait per hardware
instruction: inputs arrive via three packed DMAs with per-engine fan-in NOPs,
per-engine tile pools avoid cross-engine slot joins, and _split_waits hoists
any residual multi-waits onto same-engine NoOps.
"""

import threading

import numpy as np
import ml_dtypes

B, N, D, H = 4, 512, 128, 128
NCORES = 8
S_FOLD = 64.0          # fp8 range scale folded into |W2c|; sigmoid applies 1/S

# pk16 bf16 pack columns (preamble-critical data first: it rides DMA A)
C_XT = 0               # [0,512)    X_b.T
C_W1B = 512            # [512,640)  W1c[D:]*|W2c|*S
C_W1A = 640            # [640,768)  W1c[:D]*|W2c|*S
C_XIT = 768            # [768,1024) own 256 rows transposed (block order)
C_BEFFC = 1024         # b1c*|W2c|*S
C_B1SE = 1025          # b1s*|W2s|*S
C_B2C = 1026           # b2c broadcast
C_B2S = 1027           # b2s broadcast
PK16_A = 1028          # end of DMA A
C_W1S = 1028           # [1028,1156) W1s*|W2s|*S
C_ZZ = 1156            # [1156,1412) sliding sign window (col 128 = sgn)
C_ZS = 1412            # [1412,1413) sign(W2s)
PK16_W = 1413

# per-class row split: (n_dve_bf16, n_dve8_pairs, n_act8_pairs, n_pool8_pairs)
# rows: n_dve + 2*(pairs) = 32.  Tuned against TimelineSim.
CFG = {
    0: (22, 0, 3, 2),   # W=512
    1: (16, 1, 4, 3),   # W=448
    2: (20, 0, 2, 4),   # W=384
    3: (20, 0, 4, 2),   # W=320
    4: (20, 0, 2, 4),   # W=256
    5: (16, 1, 4, 3),   # W=192
    6: (20, 0, 3, 3),   # W=128
    7: (20, 0, 3, 3),   # W=64
}

_lock = threading.Lock()
_cache = {}


def _build_nc():
    import concourse.bass as bass
    import concourse.mybir as mybir
    import concourse.tile as tile
    from concourse.tile import add_dep_helper

    fp32 = mybir.dt.float32
    bf16 = mybir.dt.bfloat16
    fp8 = mybir.dt.float8e4

    nc = bass.Bass("TRN2")
    pk16 = nc.dram_tensor("pk16", [128, PK16_W], bf16, kind="ExternalInput")
    pk8 = nc.dram_tensor("pk8", [128, 2, 256], fp8, kind="ExternalInput")
    va = nc.dram_tensor("va", [128, 512], bf16, kind="ExternalOutput")
    vb = nc.dram_tensor("vb", [128, 256], bf16, kind="ExternalOutput")
    dd = nc.dram_tensor("dd", [1, 256], fp32, kind="ExternalOutput")

    exit_prods = []

    def spnop(prod):
        n = nc.sync.nop(nofuse=True)
        add_dep_helper(n.ins, prod.ins, sync=True, reason="sp fanin")
        return n

    with tile.TileContext(nc) as tc:
        with (
            tc.tile_pool(name="singles", bufs=1) as singles,
            tc.tile_pool(name="mdve", bufs=12) as mdve,
            tc.tile_pool(name="mact", bufs=8) as mact,
            tc.tile_pool(name="mpool", bufs=8) as mpool,
            tc.tile_pool(name="pre_ps", bufs=1, space="PSUM") as pre_ps,
            tc.tile_pool(name="acc0_ps", bufs=1, space="PSUM") as acc0_ps,
            tc.tile_pool(name="acc1_ps", bufs=1, space="PSUM") as acc1_ps,
        ):
            p16 = singles.tile([128, PK16_W], bf16, tag="p16")
            dmaA = nc.sync.dma_start(out=p16[:, 0:PK16_A], in_=pk16[:, 0:PK16_A])
            dmaB = nc.scalar.dma_start(out=p16[:, PK16_A:], in_=pk16[:, PK16_A:])
            p8 = singles.tile([128, 2, 256], fp8, tag="p8")
            dma8 = nc.gpsimd.dma_start(out=p8, in_=pk8[:, :, :])
            exit_prods += [dmaA, dmaB, dma8]

            xt = p16[:, C_XT:C_XT + 512]
            xit = p16[:, C_XIT:C_XIT + 256]
            w1a = p16[:, C_W1A:C_W1A + 128]
            w1b = p16[:, C_W1B:C_W1B + 128]
            w1s = p16[:, C_W1S:C_W1S + 128]
            zz = p16[:, C_ZZ:C_ZZ + 256]
            zs = p16[:, C_ZS:C_ZS + 1]
            scal = singles.tile([128, 4], fp32, tag="scal")
            nc.vector.tensor_copy(out=scal, in_=p16[:, C_BEFFC:C_BEFFC + 4])
            beffc = scal[:, 0:1]
            b1se = scal[:, 1:2]
            b2c = scal[:, 2:3]
            b2s = scal[:, 3:4]
            t8 = p8

            # --- c2 = (w1b.T @ xt) [H, 512] bf16 (xt pre-rolled for half B) ---
            cpsum = pre_ps.tile([H, 512], fp32, tag="cpsum")
            nc.tensor.matmul(cpsum, w1b, xt, start=True, stop=True)
            c2 = singles.tile([H, 512], bf16, tag="c2")
            nc.vector.tensor_copy(out=c2, in_=cpsum)

            # --- abias = (w1a.T @ xit) + beffc  [H, 256] fp32 ---
            apsum = pre_ps.tile([H, 256], fp32, tag="apsum")
            nc.tensor.matmul(apsum, w1a, xit, start=True, stop=True)
            abias = singles.tile([H, 256], fp32, tag="abias")
            nc.vector.tensor_scalar(out=abias, in0=apsum, scalar1=beffc,
                                    scalar2=None, op0=mybir.AluOpType.add)

            # --- main: 8 width classes x 32 rows, mixed bf16/fp8-DR ---
            ps0 = acc0_ps.tile([128, 512], fp32, tag="ps0")
            ps1 = acc1_ps.tile([128, 256], fp32, tag="ps1")

            last_gen = last_mm = None
            for q in range(8):
                W = 512 - 64 * q
                j0 = 64 * q
                # Device always reads window [j0, j0+W) of c2; half B's true
                # window is [j0+32, j0+32+W), handled by pre-rolling X's
                # columns on the host so the instruction stream is uniform.
                ps = ps0 if q < 4 else ps1
                pr = 32 * (q % 4)      # psum row base
                n_dve, n_dve8, n_act8, n_pool8 = CFG[q]
                # one long accumulation group per psum tile (classes 0-3 in
                # ps0, 4-7 in ps1): every matmul writes all 128 partitions
                # (one-hot row is global), so per-class start would wipe
                # earlier classes' has_written state.
                grp_start = (q % 4 == 0)
                grp_stop = (q % 4 == 3)
                n_mm = n_dve + n_dve8 + n_act8 + n_pool8
                mm_idx = 0

                def emit_mm(lhsT, rhs, dr):
                    nonlocal mm_idx, last_mm
                    last_mm = nc.tensor.matmul(
                        ps[:, 0:W], lhsT, rhs,
                        start=(grp_start and mm_idx == 0),
                        stop=(grp_stop and mm_idx == n_mm - 1),
                        perf_mode=(mybir.MatmulPerfMode.DoubleRow if dr else None),
                        skip_group_check=True)
                    mm_idx += 1

                # task list: interleave fp8 pairs evenly among bf16 singles
                # so no engine's tiles bunch up at the block's end
                pairs = (["dve8"] * n_dve8 + ["act8"] * n_act8
                         + ["pool8"] * n_pool8)
                tasks = ["dve"] * n_dve
                np_ = len(pairs)
                for k, pk in enumerate(pairs):
                    pos = min(len(tasks), (k + 1) * (n_dve + np_) // (np_ + 1))
                    tasks.insert(min(pos, len(tasks)), pk)

                mc = 0
                for tsk in tasks:
                    il = 32 * q + mc
                    if tsk == "dve":
                        m = mdve.tile([H, W], bf16, tag="mdve")
                        last_gen = nc.vector.tensor_scalar(
                            out=m, in0=c2[:, j0:j0 + W],
                            scalar1=abias[:, il:il + 1], scalar2=0.0,
                            op0=mybir.AluOpType.add, op1=mybir.AluOpType.max)
                        mcg = pr + mc
                        emit_mm(zz[:, 128 - mcg:256 - mcg], m, dr=False)
                        mc += 1
                        continue
                    if tsk == "dve8":
                        m8 = mdve.tile([H, 2, W], fp8, tag="m8dve")
                        eng = nc.vector
                    elif tsk == "act8":
                        m8 = mact.tile([H, 2, W], fp8, tag="m8act")
                        eng = None
                    else:
                        m8 = mpool.tile([H, 2, W], fp8, tag="m8pool")
                        eng = nc.gpsimd
                    for z in range(2):
                        if tsk == "act8":
                            last_gen = nc.scalar.activation(
                                out=m8[:, z, :], in_=c2[:, j0:j0 + W],
                                func=mybir.ActivationFunctionType.Relu,
                                bias=abias[:, il + z:il + z + 1], scale=1.0)
                        else:
                            last_gen = eng.tensor_scalar(
                                out=m8[:, z, :], in0=c2[:, j0:j0 + W],
                                scalar1=abias[:, il + z:il + z + 1],
                                scalar2=0.0,
                                op0=mybir.AluOpType.add,
                                op1=mybir.AluOpType.max)
                    mcg = pr + mc
                    emit_mm(t8[:, :, 128 - mcg:256 - mcg], m8, dr=True)
                    mc += 2

                if q == 3:
                    # self-sim diagonal (off the critical path)
                    spsum = pre_ps.tile([H, 256], fp32, tag="spsum")
                    nc.tensor.matmul(spsum, w1s, xit, start=True, stop=True)
                    ms = singles.tile([H, 256], bf16, tag="ms")
                    nc.vector.tensor_scalar(
                        out=ms, in0=spsum, scalar1=b1se, scalar2=0.0,
                        op0=mybir.AluOpType.add, op1=mybir.AluOpType.max)
                    dpsum = pre_ps.tile([1, 256], fp32, tag="dpsum")
                    nc.tensor.matmul(dpsum, zs, ms, start=True, stop=True)
                    dsig = singles.tile([1, 256], fp32, tag="dsig")
                    act_d = nc.scalar.activation(
                        out=dsig, in_=dpsum,
                        func=mybir.ActivationFunctionType.Sigmoid,
                        bias=b2s[0:1, :], scale=1.0 / S_FOLD)
                    spnop(act_d)
                    exit_prods.append(nc.sync.dma_start(out=dd[:, :], in_=dsig))
                    # ps0 complete: emit its sigmoid now so ACT's FIFO can
                    # run it while classes 4-7 generate
                    v0 = singles.tile([128, 512], bf16, tag="v0")
                    act0 = nc.scalar.activation(
                        out=v0, in_=ps0,
                        func=mybir.ActivationFunctionType.Sigmoid,
                        bias=b2c, scale=1.0 / S_FOLD)
                    spnop(act0)
                    exit_prods.append(nc.sync.dma_start(out=va[:, :], in_=v0))

            # --- sigmoid + output for ps1 ---
            v1 = singles.tile([128, 256], bf16, tag="v1")
            act1 = nc.scalar.activation(out=v1, in_=ps1,
                                        func=mybir.ActivationFunctionType.Sigmoid,
                                        bias=b2c, scale=1.0 / S_FOLD)
            spnop(act1)
            exit_prods.append(nc.sync.dma_start(out=vb[:, :], in_=v1))

            exit_prods += [last_gen, last_mm]
            for p in exit_prods:
                if p is not None:
                    spnop(p)
    return nc


def _split_waits(bir_bytes):
    """Post-pass: walrus on this image accepts ~1 sem wait per instruction.
    Hoist all-but-one wait of any multi-wait instruction onto same-engine
    NoOps inserted immediately before it (engine stalls on each in order --
    semantically identical, within the 1-wait limit)."""
    import json
    bir = json.loads(bir_bytes)
    counter = [0]

    def mknop(engine, wait, debug):
        counter[0] += 1
        return {
            "debug": debug,
            "engine": engine,
            "ins": [],
            "name": f"WSN-{counter[0]}",
            "opcode": "NoOp",
            "outs": [],
            "sync_info": {"on_update": [], "on_wait": [wait]},
        }

    def process(blocks):
        for blk in blocks:
            insts = blk.get("instructions")
            if not insts:
                continue
            out = []
            for ins in insts:
                si = ins.get("sync_info")
                ow = (si or {}).get("on_wait") or []
                if len(ow) > 1:
                    for w in ow[:-1]:
                        out.append(mknop(ins["engine"], w, ins.get("debug", 0)))
                    si["on_wait"] = [ow[-1]]
                out.append(ins)
            blk["instructions"] = out

    for func in bir.get("functions", []):
        process(func.get("blocks", []))
    return json.dumps(bir).encode()


def _get_nc():
    with _lock:
        if "nc" not in _cache:
            nc = _build_nc()
            orig = nc.to_json_bytes
            nc.to_json_bytes = lambda: _split_waits(orig())
            _cache["nc"] = nc
        return _cache["nc"]


def _row_of(q, half, r):
    """Global row index of local row r in class q for the given half."""
    return 64 * q + 32 * half + r


def make_core_inputs(X, W1c, b1c, W2c, b2c, W1s, b1s, W2s, b2s):
    """Build the 8 per-core input maps (host-side weight folding)."""
    X = np.ascontiguousarray(np.asarray(X, np.float32))
    w2c = np.asarray(W2c, np.float32).reshape(-1)
    w2s = np.asarray(W2s, np.float32).reshape(-1)
    aw = np.abs(w2c) * S_FOLD
    sg = np.sign(w2c).astype(np.float32)
    aws = np.abs(w2s) * S_FOLD
    sgs = np.sign(w2s).astype(np.float32)
    W1c = np.asarray(W1c, np.float32)

    base16 = np.zeros((128, PK16_W), np.float32)
    base16[:, C_W1A:C_W1A + 128] = W1c[:D] * aw[None, :]
    base16[:, C_W1B:C_W1B + 128] = W1c[D:] * aw[None, :]
    base16[:, C_W1S:C_W1S + 128] = np.asarray(W1s, np.float32) * aws[None, :]
    base16[:, C_ZZ + 128] = sg
    base16[:, C_ZS] = sgs

    base16[:, C_BEFFC] = np.asarray(b1c, np.float32) * aw
    base16[:, C_B1SE] = np.asarray(b1s, np.float32) * aws
    base16[:, C_B2C] = float(np.asarray(b2c).reshape(-1)[0])
    base16[:, C_B2S] = float(np.asarray(b2s).reshape(-1)[0])

    p8 = np.zeros((128, 2, 256), np.float32)
    p8[:, 0, 128] = sg
    p8[:, 1, 129] = sg
    p8 = p8.astype(ml_dtypes.float8_e4m3)

    in_maps = []
    for c in range(NCORES):
        b, half = c // 2, c % 2
        p16 = base16.copy()
        # xt: device reads window [64q, 64q+W) of c2 for class q.  For half B
        # the true window is [64q+32, 64q+32+W): roll X columns left by 32 so
        # the same device offsets pick up the shifted window; the final 32
        # (wrapped) columns fall in the junk region of every class and are
        # discarded by the host.
        xb = X[b].T  # [D, N]
        if half == 1:
            xb = np.roll(xb, -32, axis=1)
        p16[:, C_XT:C_XT + 512] = xb
        rows = np.concatenate(
            [np.arange(64 * q + 32 * half, 64 * q + 32 * half + 32)
             for q in range(8)])
        p16[:, C_XIT:C_XIT + 256] = X[b, rows].T
        in_maps.append({"pk16": p16.astype(ml_dtypes.bfloat16), "pk8": p8})
    return in_maps


def assemble(results, dtype=np.float32):
    """results: list of 8 dicts with va [128,512] bf16, vb [128,256] bf16,
    dd [1,256] f32."""
    sim = np.zeros((B, N, N), np.float32)
    for b in range(B):
        U = np.zeros((N, N), np.float32)
        diag = np.zeros(N, np.float32)
        for half in range(2):
            r = results[2 * b + half]
            v0 = np.asarray(r["va"], np.float32)
            v1 = np.asarray(r["vb"], np.float32)
            d = np.asarray(r["dd"]).reshape(256)
            for q in range(8):
                W = 512 - 64 * q
                j0 = 64 * q + 32 * half
                wv = 512 - j0          # valid width
                g0 = 64 * q + 32 * half
                v = v0 if q < 4 else v1
                pr = 32 * (q % 4)
                U[g0:g0 + 32, j0:j0 + wv] = v[pr:pr + 32, 0:wv]
                diag[g0:g0 + 32] = d[32 * q:32 * q + 32]
        Ut = np.triu(U, 1)
        out = Ut + Ut.T
        np.fill_diagonal(out, diag)
        sim[b] = out
    return sim.astype(dtype)


def kernel(X, W1c, b1c, W2c, b2c, W1s, b1s, W2s, b2s):
    from concourse.bass_utils import run_bass_kernel_spmd

    nc = _get_nc()
    in_maps = make_core_inputs(X, W1c, b1c, W2c, b2c, W1s, b1s, W2s, b2s)
    res = run_bass_kernel_spmd(nc, in_maps, core_ids=list(range(NCORES)))
    return assemble(res.results, dtype=np.asarray(X).dtype)
